# revision 8
# baseline (speedup 1.0000x reference)
"""Embedding lookup (gather) on 8 TRN2 NeuronCores.

Strategy (per the row-sharding hint): the 1M x 128 table is row-sharded by
value range -- core c owns rows [c*125000, (c+1)*125000), held as 4 windows
of 31250 rows so window-local indices fit int16. The host routes each of the
500K indices to its owning window (the sharding step) and the device gathers
rows with batched-descriptor SWDGE gathers (InstDMAGatherAnt, ~0.34ns/row
descriptor) instead of one indirect DMA per 128 rows (~1.5us fixed cost
each). Unsharding re-assembles rows into token order on the host (inverse of
the routing permutation).

The table is fed to the device as bf16 (max rounding error 2^-8 ~ 0.4%,
well inside the 2e-2 gate) which halves both the gathered-read and
write-back HBM traffic; the host upcasts the result to fp32.

Window sections are capacity-padded with dummy index 0 so every gather's
descriptor count is static. num_idxs per dma_gather is capped at 1024 by the
ucode's index-read pattern (HW-probed; larger values crash the device), so
each window is gathered in 16 chunks rotated across the 4 SWDGE queues.

Any token overflowing its window's capacity (impossible in practice for
uniform indices: capacity is mean + ~5 sigma) is gathered on the host, so
correctness never depends on the index distribution.
"""
import sys
import numpy as np

sys.path.insert(0, "/opt/trn_rl_repo")

import ml_dtypes

import concourse.bacc as bacc
import concourse.bass as bass
import concourse.mybir as mybir
import concourse.tile as tile
from concourse import bass_utils

N_EMB = 1_000_000
D = 128
N_IDX = 500_000
N_CORES = 8

W_ROWS = 31_250              # rows per window (< 2**15 so int16 indexes work)
WIN_PER_CORE = 4
CORE_ROWS = W_ROWS * WIN_PER_CORE      # 125000 table rows owned per core

# HW-probed limit: dma_gather works at num_idxs=1024 and crashes the device
# at 1152+ (the ucode's index-read pattern tops out at 64 int16 columns per
# partition), so one gather instruction moves at most 1024 rows.
IDX_PER_GATHER = 1024
CHUNKS_PER_WIN = 16
CAP = IDX_PER_GATHER * CHUNKS_PER_WIN  # 16384 token slots per window section
NTILE = WIN_PER_CORE * CHUNKS_PER_WIN  # gather chunks per core
IDX_COLS = IDX_PER_GATHER // 16        # int16 idx columns per partition
GCOLS = -(-IDX_PER_GATHER // 128)      # dst free-dim row groups (cdiv)

DTYPE = mybir.dt.bfloat16
NP_DTYPE = ml_dtypes.bfloat16

_cached = None


def _build():
    global _cached
    if _cached is not None:
        return _cached

    nc = bacc.Bacc(
        "TRN2",
        target_bir_lowering=False,
        debug=False,
        enable_asserts=False,
        num_devices=N_CORES,
        num_swdge_queues=4,
    )
    # int16 window-local indices, 16-wrapped (token i of a chunk at partition
    # i%16, column i//16) and replicated to all 8 gpsimd cores' partitions.
    idx16 = nc.dram_tensor(
        "idx16", [128, NTILE * IDX_COLS], mybir.dt.int16, kind="ExternalInput"
    ).ap()
    wsh = nc.dram_tensor(
        "wsh", [CORE_ROWS, D], DTYPE, kind="ExternalInput"
    ).ap()
    out = nc.dram_tensor(
        "out", [NTILE, 128, GCOLS, D], DTYPE, kind="ExternalOutput"
    ).ap()

    with tile.TileContext(nc) as tc:
        with (
            tc.tile_pool(name="idxp", bufs=1) as idxp,
            tc.tile_pool(name="pool", bufs=4) as pool,
        ):
            idx_all = idxp.tile([128, NTILE * IDX_COLS], mybir.dt.int16)
            nc.sync.dma_start(out=idx_all[:, :], in_=idx16[:, :])
            for k in range(NTILE):
                w = k // CHUNKS_PER_WIN
                g = pool.tile([128, GCOLS, D], DTYPE, tag="g")
                # One instruction gathers this chunk's rows:
                # row i of the chunk -> g[i%128, i//128, :].
                nc.gpsimd.dma_gather(
                    g[:, :, :],
                    wsh[w * W_ROWS:(w + 1) * W_ROWS, :],
                    idx_all[:, k * IDX_COLS:(k + 1) * IDX_COLS],
                    IDX_PER_GATHER,   # num_idxs
                    IDX_PER_GATHER,   # num_idxs_reg: static, all slots valid
                    D,                # elem_size (elements per row)
                    elem_step=D,
                    queue_num=k % 4,
                    # One packet per descriptor: packets are the SDMA queue-
                    # switch boundary, so per-desc packets let each engine
                    # interleave the 4 queues' random HBM reads.
                    single_packet=False,
                )
                wb = nc.sync if k % 2 == 0 else nc.scalar
                wb.dma_start(out=out[k], in_=g[:, :, :])

    nc.compile()
    _cached = nc
    return nc


def make_feeds(input, weight):
    """Route tokens to (core, window, slot); build per-core device feeds.

    Returns (in_maps, flat_slot_of_token, host_idx) where flat_slot_of_token
    maps token t to its row in the concatenated device outputs (-1 if the
    token overflowed its window and must be host-gathered from host_idx).
    """
    idx = np.asarray(input).astype(np.int64).ravel()
    assert idx.shape == (N_IDX,)
    w = np.asarray(weight).astype(NP_DTYPE)

    ws = idx // W_ROWS                      # global window id, 0..31
    lo = (idx % W_ROWS).astype(np.int16)    # window-local row

    order = np.argsort(ws, kind="stable")
    ws_sorted = ws[order]
    counts = np.bincount(ws, minlength=N_CORES * WIN_PER_CORE)
    starts = np.zeros(N_CORES * WIN_PER_CORE + 1, dtype=np.int64)
    np.cumsum(counts, out=starts[1:])
    rank = np.arange(N_IDX, dtype=np.int64) - starts[ws_sorted]
    valid = rank < CAP                      # overflow -> host fallback

    slot_global = ws_sorted * CAP + rank    # slot in [32 * CAP) padded space
    lo_slots = np.zeros(N_CORES * WIN_PER_CORE * CAP, dtype=np.int16)
    lo_slots[slot_global[valid]] = lo[order][valid]

    # Wrap-16 + replicate-to-128-partitions feed layout per chunk.
    wrapped = lo_slots.reshape(N_CORES, NTILE, IDX_COLS, 16).transpose(0, 1, 3, 2)
    feed = np.broadcast_to(
        wrapped.reshape(N_CORES, NTILE, 1, 16, IDX_COLS),
        (N_CORES, NTILE, 8, 16, IDX_COLS),
    ).reshape(N_CORES, NTILE, 128, IDX_COLS).transpose(0, 2, 1, 3)
    idx16_feed = np.ascontiguousarray(
        feed.reshape(N_CORES, 128, NTILE * IDX_COLS)
    )

    in_maps = [
        {
            "idx16": idx16_feed[c],
            "wsh": np.ascontiguousarray(w[c * CORE_ROWS:(c + 1) * CORE_ROWS]),
        }
        for c in range(N_CORES)
    ]

    # Device row of slot s (core c, chunk k, i = s % IDX_PER_GATHER):
    # concatenated-out flat row ((c*NTILE + k)*128 + i%128)*GCOLS + i//128
    sg = slot_global
    c_ = sg // (NTILE * IDX_PER_GATHER)
    s_ = sg % (NTILE * IDX_PER_GATHER)
    k_ = s_ // IDX_PER_GATHER
    i_ = s_ % IDX_PER_GATHER
    flat_sorted = ((c_ * NTILE + k_) * 128 + i_ % 128) * GCOLS + i_ // 128
    flat_slot_of_token = np.full(N_IDX, -1, dtype=np.int64)
    flat_slot_of_token[order[valid]] = flat_sorted[valid]
    return in_maps, flat_slot_of_token, idx


def kernel(input, weight, _trace=False, _tmpdir=None):
    nc = _build()
    in_maps, flat_slot, idx = make_feeds(input, weight)

    res = bass_utils.run_bass_kernel_spmd(
        nc,
        in_maps,
        core_ids=list(range(N_CORES)),
        trace=_trace,
        tmpdir=_tmpdir,
    )

    allrows = np.concatenate(
        [
            np.asarray(res.results[c]["out"]).reshape(NTILE * 128 * GCOLS, D)
            for c in range(N_CORES)
        ],
        axis=0,
    )
    missing = flat_slot < 0
    out = allrows[np.where(missing, 0, flat_slot)].astype(np.float32)
    if missing.any():
        wfull = np.asarray(weight, dtype=np.float32)
        out[missing] = wfull[idx[missing]]
    if _trace:
        return out, res
    return out
